# revision 3
# baseline (speedup 1.0000x reference)
"""Hadamard transform kernel for Trainium2 (8 NeuronCores, SPMD data-parallel).

Math (per core, shard of 2048 rows, input xT [1024, 2048] fp16):
  H_1024 = H_8 (x) H_128 with d = a*128 + b.  Slab a = xT rows
  [a*128,(a+1)*128).  The H_8 factor is three butterfly stages; stages
  1-2 run on DVE (tensor_tensor, 2x fp16 mode) with a few ops on
  GPSIMD (scalar_tensor_tensor — cheaper than tensor_tensor there),
  and stage 3 is folded into the PE pass: each 128-col output region
  accumulates lhsT=u_even then lhsT=u_odd against rhs = +R / -R
  (R = H_128^T / 32 fp16, folding the 1/sqrt(1024) scale).

Schedule (vs v1's 32.2us):
  * PE warmup chain of dummy matmuls keeps the tensor-engine p-state
    ramp satisfied so real matmuls run at full clock (53ns vs 107ns).
  * Butterfly ops gated on the last-loaded slab pair are emitted in
    column chunks (separate tiles) so block-0's matmuls start ~10us in
    and the first out-DMA hits the wire right as the in-stream drains.
  * Matmuls per 128-row block go into one PSUM [128,1024] tile, L-side
    regions emitted `r_lag` blocks ahead of R-side ones (GPSIMD's
    slower u4/u6 feed only R regions).
  * Lag-free drain: one wide downcast copy per block (ACT or DVE per
    tuned map, or split), out-DMA fires immediately after.  h1's
    butterflies are spliced into the DVE queue between h0's drain
    copies per a tuned slot map.
"""

import numpy as np
from contextlib import ExitStack

import concourse.bass as bass
import concourse.tile as tile
from concourse import bacc, bass_utils, mybir

N_CORES = 8
B, S, D = 4, 4096, 1024
ROWS = B * S
SHARD = ROWS // N_CORES      # 2048
HALF = SHARD // 2            # 1024 cols (rows of x) per half
F32 = mybir.dt.float32
F16 = mybir.dt.float16
ALU = mybir.AluOpType

_cache = {}

CFG = {
    # slab-pair load order per half (entry i loads slabs (i, i+4))
    "load_order": ((0, 2, 1, 3), (0, 2, 1, 3)),
    "warmup_n": 18,
    "warmup_free": 512,
    # butterfly ops on Pool instead of DVE: (h, opname[, chunk])
    "pool_ops": ((0, "t4"), (0, "t6"), (0, "u4"), (0, "u6"),
                 (1, "t4"), (1, "t6"), (1, "u4", 1), (1, "u6", 1)),
    # non-gated ops also emitted in column chunks (feeds arrive earlier)
    "chunk_ops": ((0, "t4"), (0, "t6"), (0, "u4"), (0, "u6"),
                  (1, "t4"), (1, "t6"), (1, "u4"), (1, "u6")),
    # gated-op chunk width per half (0 = full width)
    "chunk_w": (512, 512),
    # copy engine per block 0..15: a=ACT wide, d=DVE wide, s=split A/D
    "copy_eng": "aaaaaadaaadaadad",
    # L-side emitted this many blocks ahead of R-side (per half)
    "r_lag": (0, 0),
    # tokens emitted before the h0 block loop, in order ("h:name",
    # "h:name@chunk", gated passes "h:ga"/"h:gb" and splits ga1/ga2...)
    "pre_plan": ("0:t0", "0:t4@0", "0:t2", "0:t6@0", "0:u0", "0:u4@0",
                 "0:u2", "0:u6@0", "0:t1", "0:t5", "0:t4@1", "0:t6@1",
                 "0:u4@1", "0:u6@1", "0:ga",
                 "1:t0", "1:t2", "0:gb", "1:t1", "1:t5",
                 "1:t4@0", "1:t6@0", "1:t4@1", "1:t6@1"),
    # remaining tokens -> emit after h0 block idx's drain
    "h1_slots": {"1:ga1": 0, "1:u0": 1, "1:u2": 2, "1:ga2": 3,
                 "1:u4@0": 4, "1:u6@0": 5, "1:gb1": 6, "1:gb2": 7},
    "ps_bufs": 4,
}



def _build_nc(cfg=None):
    cfg = {**CFG, **(cfg or {})}
    pool_ops = set(map(tuple, cfg["pool_ops"]))
    nc = bacc.Bacc("TRN2", target_bir_lowering=False, debug=False)
    xt_d = nc.dram_tensor("xt", [D, SHARD], F16, kind="ExternalInput").ap()
    r_d = nc.dram_tensor("r", [128, 128], F16, kind="ExternalInput").ap()
    o_d = nc.dram_tensor("out", [SHARD, D], F16, kind="ExternalOutput").ap()

    with tile.TileContext(nc) as tc, ExitStack() as ctx:
        const_pool = ctx.enter_context(tc.tile_pool(name="const", bufs=1))
        x_pool = ctx.enter_context(tc.tile_pool(name="x", bufs=1))
        t_pool = ctx.enter_context(tc.tile_pool(name="t", bufs=1))
        u_pool = ctx.enter_context(tc.tile_pool(name="u", bufs=1))
        y_pool = ctx.enter_context(tc.tile_pool(name="y", bufs=1))
        wu_pool = ctx.enter_context(tc.tile_pool(name="wu", bufs=1))
        ps_pool = ctx.enter_context(
            tc.tile_pool(name="ps", bufs=cfg["ps_bufs"], space="PSUM"))

        # ---- PE warmup (psum tile shares the yps ring; its slot is
        # naturally reused once the warmups have drained) ----
        wu_sb = wu_pool.tile([128, cfg["warmup_free"]], F16, tag="wu",
                             name="wu")
        nc.gpsimd.memset(wu_sb[:], 0.0)
        wps = ps_pool.tile([128, D], F32, tag="yps", name="wps")
        for _ in range(cfg["warmup_n"]):
            nc.tensor.matmul(wps[:16, :cfg["warmup_free"]],
                             lhsT=wu_sb[:, :16], rhs=wu_sb[:],
                             start=True, stop=True)

        # ---- input DMAs (SP queue, program order = pipe order) ----
        x_sb = [[None] * 8 for _ in range(2)]

        def load_half(h):
            c0 = h * HALF
            for i in cfg["load_order"][h]:
                for a in (i, i + 4):
                    xs = x_pool.tile([128, HALF], F16, tag=f"x{a}_{h}",
                                     name=f"x{a}_{h}")
                    nc.sync.dma_start(
                        xs[:], xt_d[a * 128:(a + 1) * 128, c0:c0 + HALF])
                    x_sb[h][a] = xs

        load_half(0)
        R_sb = const_pool.tile([128, 128], F16, tag="R", name="R")
        nc.sync.dma_start(R_sb[:], r_d[:])
        load_half(1)

        # Rn = -R on ACT (idle early; off DVE's critical queue)
        Rn_sb = const_pool.tile([128, 128], F16, tag="Rn", name="Rn")
        nc.scalar.activation(Rn_sb[:], R_sb[:],
                             mybir.ActivationFunctionType.Copy, scale=-1.0)

        # ---- butterfly helpers ----
        chunks = {}

        def op_tiles(h, name, w_chunk):
            pool = t_pool if name[0] == "t" else u_pool
            widths = (w_chunk if isinstance(w_chunk, (tuple, list))
                      else ([w_chunk] * (HALF // w_chunk) if w_chunk else []))
            if widths and sum(widths) == HALF and len(widths) > 1:
                lst = []
                c0 = 0
                for w in widths:
                    tl_ = pool.tile([128, w], F16,
                                    tag=f"{name}_{h}_{c0}",
                                    name=f"{name}_{h}_{c0}")
                    lst.append((tl_, c0, c0 + w))
                    c0 += w
                chunks[(h, name)] = lst
            else:
                tl_ = pool.tile([128, HALF], F16, tag=f"{name}_{h}",
                                name=f"{name}_{h}")
                chunks[(h, name)] = [(tl_, 0, HALF)]
            return chunks[(h, name)]

        def rd(h, name, c0, c1):
            for tl_, a, b in chunks[(h, name)]:
                if a <= c0 and c1 <= b:
                    return tl_[:, c0 - a:c1 - a]
            raise KeyError((h, name, c0, c1))

        def emit_op(h, name, src0, src1, sub, w_chunk=0, chunk_i=None):
            """name = src0 + src1, or src0 - src1 (sub=True).
            chunk_i: emit only that chunk index (tiles allocated once)."""
            tiles = (chunks.get((h, name))
                     or op_tiles(h, name, w_chunk))
            for ci, (tl_, c0, c1) in enumerate(tiles):
                if chunk_i is not None and ci != chunk_i:
                    continue
                on_pool = ((h, name) in pool_ops
                           or (h, name, ci) in pool_ops)

                def ap(s):
                    if s[0] == "x":
                        return x_sb[h][s[1]][:, c0:c1]
                    return rd(h, s[1], c0, c1)
                if on_pool:
                    # TensorScalarPtr is not a legal Pool opcode on HW
                    # (walrus rejects it) — plain tensor_tensor only
                    if sub:
                        nc.gpsimd.tensor_sub(tl_[:], ap(src0), ap(src1))
                    else:
                        nc.gpsimd.tensor_add(tl_[:], ap(src0), ap(src1))
                elif sub:
                    nc.vector.tensor_sub(tl_[:], ap(src0), ap(src1))
                else:
                    nc.vector.tensor_add(tl_[:], ap(src0), ap(src1))

        # s1: t_i = x_i + x_{i+4}; t_{i+4} = x_i - x_{i+4}
        # s2: u_{g+i} = t_{g+i} + t_{g+i+2}; u_{g+i+2} = t_{g+i} - t_{g+i+2}
        def op_spec():
            spec = {}
            for i in range(4):
                spec[f"t{i}"] = (("x", i), ("x", i + 4), False)
                spec[f"t{i + 4}"] = (("x", i), ("x", i + 4), True)
            for g in (0, 4):
                for i in (0, 1):
                    spec[f"u{g + i}"] = (
                        ("t", f"t{g + i}"), ("t", f"t{g + i + 2}"), False)
                    spec[f"u{g + i + 2}"] = (
                        ("t", f"t{g + i}"), ("t", f"t{g + i + 2}"), True)
            return spec

        SPEC = op_spec()

        def h_order(o):
            a, b, c, g = o
            return [f"t{a}", f"t{a + 4}", f"t{b}", f"t{b + 4}",
                    "u0", "u2", "u4", "u6",
                    f"t{c}", f"t{c + 4}",
                    f"t{g}", f"t{g + 4}", "u1", "u3", "u5", "u7"]

        def gated_ops(o):
            g = o[3]
            return {f"t{g}", f"t{g + 4}", "u1", "u3", "u5", "u7"}

        def emit_named(h, name, chunk_i=None):
            s0, s1_, sub = SPEC[name]
            wch = cfg["chunk_w"][h]
            g = gated_ops(cfg["load_order"][h])
            chunked = (name in g or
                       (h, name) in set(map(tuple, cfg.get("chunk_ops", ()))))
            emit_op(h, name, s0, s1_, sub, wch if chunked else 0,
                    chunk_i=chunk_i)

        def emit_gated(h):
            """Gated ops in chunk-major order: all chunk-0 ops first, so
            the first row blocks' matmuls can start before chunk 1 runs."""
            o = cfg["load_order"][h]
            g = o[3]
            names = [f"t{g}", f"t{g + 4}", "u1", "u3", "u5", "u7"]
            wch = cfg["chunk_w"][h]
            nch = HALF // wch if wch else 1
            for ci in range(nch):
                for name in names:
                    emit_named(h, name, chunk_i=ci if wch else None)

        # ---- token emission ----
        def gated_names(h):
            g = cfg["load_order"][h][3]
            return [f"t{g}", "u1", "u3", f"t{g + 4}", "u5", "u7"]

        def n_chunks(h, name):
            wch = cfg["chunk_w"][h]
            chunked = (name in gated_names(h) or
                       (h, name) in set(map(tuple, cfg.get("chunk_ops", ()))))
            if not (chunked and wch):
                return 1
            return (len(wch) if isinstance(wch, (tuple, list))
                    else HALF // wch)

        def atoms_of(tok):
            """Expand a token to [(h, name, ci)] atoms."""
            h, item = tok.split(":")
            h = int(h)
            gn = gated_names(h)
            if "@" in item:
                name, ci = item.split("@")
                return [(h, name, int(ci))]
            if item.startswith("gp"):
                body = item[2:]
                side = body[-1] if body[-1] in "LR" else ""
                ci = int(body[:-1] if side else body)
                if ci >= n_chunks(h, gn[0]):
                    return []
                sel = gn[:3] if side == "L" else gn[3:] if side == "R" else gn
                return [(h, n, ci) for n in sel]
            if item in ("ga", "ga1", "ga2", "gb", "gb1", "gb2"):
                ci = 0 if item[1] == "a" else n_chunks(h, gn[0]) - 1
                if ci < 0:
                    return []
                sel = gn if len(item) == 2 else (
                    gn[:3] if item[2] == "1" else gn[3:])
                return [(h, n, ci) for n in sel]
            return [(h, item, ci) for ci in range(n_chunks(h, item))]

        emitted_atoms = set()

        def emit_tok(tok):
            for h, name, ci in atoms_of(tok):
                if (h, name, ci) in emitted_atoms:
                    continue
                emit_named(h, name, chunk_i=ci)
                emitted_atoms.add((h, name, ci))

        def covers(tok):
            return all(a in emitted_atoms for a in atoms_of(tok))

        def all_tokens():
            out = []
            for h in (0, 1):
                gn = set(gated_names(h))
                for name in h_order(cfg["load_order"][h]):
                    if name not in gn:
                        out.append(f"{h}:{name}")
                out.append(f"{h}:ga")
                out.append(f"{h}:gb")
            return out

        slots = cfg["h1_slots"]
        slot_atoms = set()
        for tok in slots:
            slot_atoms.update(atoms_of(tok))
        for tok in cfg["pre_plan"]:
            emit_tok(tok)
        for tok in all_tokens():
            for a in atoms_of(tok):
                if a not in emitted_atoms and a not in slot_atoms:
                    emit_named(a[0], a[1], chunk_i=a[2])
                    emitted_atoms.add(a)

        # ---- per-block matmul sides + drain + dma ----
        ysb = {}
        yps = {}

        def emit_mm_side(h, k, g):
            c0 = (k % 8) * 128
            c1 = c0 + 128
            if k not in yps:
                yps[k] = ps_pool.tile([128, D], F32, tag="yps",
                                      name=f"yps{k}")
            yp = yps[k]
            for j in (0, 1):
                ua = rd(h, f"u{g + 2 * j}", c0, c1)
                ub = rd(h, f"u{g + 2 * j + 1}", c0, c1)
                fa0 = (g + 2 * j) * 128
                nc.tensor.matmul(yp[:, fa0:fa0 + 128],
                                 lhsT=ua, rhs=R_sb[:],
                                 start=True, stop=False)
                nc.tensor.matmul(yp[:, fa0:fa0 + 128],
                                 lhsT=ub, rhs=R_sb[:],
                                 start=False, stop=True)
                nc.tensor.matmul(yp[:, fa0 + 128:fa0 + 256],
                                 lhsT=ua, rhs=R_sb[:],
                                 start=True, stop=False)
                nc.tensor.matmul(yp[:, fa0 + 128:fa0 + 256],
                                 lhsT=ub, rhs=Rn_sb[:],
                                 start=False, stop=True)

        def emit_block_copy(k):
            ce = cfg["copy_eng"][k]
            yp = yps[k]
            ys = y_pool.tile([128, D], F16, tag=f"ysb{k}", name=f"ysb{k}")
            ysb[k] = ys
            if ce == "a":
                nc.scalar.copy(ys[:], yp[:])
            elif ce == "d":
                nc.vector.tensor_copy(ys[:], yp[:])
            elif ce == "s":
                nc.scalar.copy(ys[:, :512], yp[:, :512])
                nc.vector.tensor_copy(ys[:, 512:], yp[:, 512:])
            else:
                raise ValueError(ce)

        def emit_block_dma(k):
            row = k * 128
            nc.sync.dma_start(o_d[row:row + 128, :], ysb[k][:])

        def emit_slot(sl):
            for tok, s in slots.items():
                if s == sl and not covers(tok):
                    emit_tok(tok)

        # ---- block schedule with L/R lag ----

        def run_half(h, k0, splice):
            lag = cfg["r_lag"]
            if not isinstance(lag, int):
                lag = lag[h]
            for i in range(8 + lag):
                if i < 8:
                    emit_mm_side(h, k0 + i, 0)          # L side
                if i >= lag:
                    k = k0 + i - lag
                    emit_mm_side(h, k, 4)               # R side
                    emit_block_copy(k)
                    emit_block_dma(k)
                    if splice:
                        emit_slot(k - k0)

        run_half(0, 0, splice=True)
        for tok in all_tokens():
            if not covers(tok):
                emit_tok(tok)
        run_half(1, 8, splice=False)

    nc.compile()
    return nc


def _get_nc():
    if "nc" not in _cache:
        _cache["nc"] = _build_nc()
    return _cache["nc"]


def kernel(x, H, **_ignored):
    x = np.asarray(x)
    H = np.asarray(H, dtype=np.float32)
    nc = _get_nc()

    R = (np.ascontiguousarray(H[:128, :128].T) / 32.0).astype(np.float16)
    xf = np.ascontiguousarray(x.reshape(ROWS, D)).astype(np.float16)
    in_maps = []
    for c in range(N_CORES):
        in_maps.append({
            "xt": np.ascontiguousarray(xf[c * SHARD:(c + 1) * SHARD].T),
            "r": R,
        })

    res = bass_utils.run_bass_kernel_spmd(nc, in_maps,
                                          core_ids=list(range(N_CORES)))

    out = np.zeros((ROWS, D, 2), dtype=np.float32)
    for c in range(N_CORES):
        out[c * SHARD:(c + 1) * SHARD, :, 0] = res.results[c]["out"]
    return out.reshape(B, S, D, 2)
